# revision 8
# baseline (speedup 1.0000x reference)
"""ConvLSTM2D forward on 8 Trainium2 NeuronCores (v3).

Problem: x [8,10,256,256,8], Wx [3,3,8,4], Wh [3,3,1,4], b [4]
         -> h_last [8,256,256,1]  (ConvLSTM, keras gate order i,f,c,o;
         i/f/o hard_sigmoid, candidate+output sigmoid)

Sharding: data-parallel over batch; core b computes batch element b fully
locally (recurrent scan stays on-core, no collectives in forward).

v3 structure (v2 was 374us, v1 420us):
 - bf16 matmul operands; x packed host-side as [T, 102, tau, cg, 258];
   4 load DMAs per step (3096B lines) on the sync queue.
 - 36 matmuls/step, N=512: pair (tau, tau+4) via 2-level free AP; 9
   accumulating passes into 4 PSUM banks (double buffered).
 - NO PSUM evacuation / deinterleave DMA: the PSUM->planar gate move is
   fused into the epilogue's first pass as partition-offset engine ops:
   per (pair q, gate): DVE/GpSimd tensor_scalar applies the hard-sigmoid
   affine (PSUM [32@g*32,2,256] -> planar bf16 [32@q*32,2,256]) and ACT
   applies Sigmoid for the candidate.  (v2 lost ~10us/step to a 256KB
   single-engine SBUF->SBUF deinterleave DMA on the critical path.)
 - NO h scatter DMA: h = o*sigmoid(c) is computed by DVE/GpSimd directly
   into the halo windows of the NEXT step's x tile (10 partition-offset
   tensor_tensor ops, windows tau0/tau4 first so cg2 matmuls restart
   early).
"""

import numpy as np
import ml_dtypes

import concourse.bacc as bacc
import concourse.bass as bass
import concourse.mybir as mybir
import concourse.tile as tile
from concourse import bass_utils

F32 = mybir.dt.float32
BF16 = mybir.dt.bfloat16
AF = mybir.ActivationFunctionType
OP = mybir.AluOpType

B, T, H, W, CIN = 8, 10, 256, 256, 8
G = 4            # gates i,f,c,o
RT = 32          # output rows per tile (M = G*RT = 128)
TAU = H // RT    # 8 row tiles
HIN = RT + 2     # input rows per tile (with halo)
NCH = 9          # 8 x-channels + h
CPG = 3          # channels per contraction group
NCG = NCH // CPG # 3 channel groups
KP = HIN * CPG   # 102 partitions per rhs tile
NPAIR = TAU // 2 # 4 tau-pairs (tau, tau+4) -> N=512 matmuls
WP = W + 2       # padded width


def h_window_segments():
    """(tau, seg_lo, seg_hi, planar_part0, planar_blk) for the h halo windows.

    Window rows for tau: 32*tau-1 .. 32*tau+32 (lr 0..33) at partition 68+lr;
    segments split where the window crosses the planar block boundary.
    """
    out = []
    for tau in range(TAU):
        r0 = tau * RT - 1
        lo = max(0, -r0)
        hi = min(HIN, H - r0)
        s = lo
        while s < hi:
            blk = (r0 + s) // 128
            e = min(hi, (blk + 1) * 128 - r0)
            out.append((tau, s, e, r0 + s - blk * 128, blk))
            s = e
    return out


def pack_inputs(x, Wx, Wh):
    """Host-side repack to bf16 device layouts.

    xk[b, t, cc*34+lr, cg, tau, 1+c] = x[b, t, 32*tau-1+lr, c, 3*cg+cc]
    wb[cg, kw, cc*34+r+kh, g*32+r]   = W9[kh, kw, 3*cg+cc, g]
    """
    x = np.asarray(x, dtype=np.float32)
    W9 = np.concatenate([np.asarray(Wx, np.float32),
                         np.asarray(Wh, np.float32)], axis=2)  # [3,3,9,4]

    xk = np.zeros((B, T, KP, NCG, TAU, WP), dtype=ml_dtypes.bfloat16)
    xb = x.astype(ml_dtypes.bfloat16)
    for tau in range(TAU):
        r0 = tau * RT - 1
        lo = max(0, -r0)
        hi = min(HIN, H - r0)
        for cg in range(NCG):
            for cc in range(CPG):
                ch = cg * CPG + cc
                if ch >= CIN:
                    continue  # h channel: written on device
                xk[:, :, cc * HIN + lo:cc * HIN + hi, cg, tau, 1:W + 1] = \
                    xb[:, :, r0 + lo:r0 + hi, :, ch]

    wb = np.zeros((NCG, 3, KP, G * RT), dtype=np.float32)
    r = np.arange(RT)
    for cg in range(NCG):
        for cc in range(CPG):
            ch = cg * CPG + cc
            for kh in range(3):
                for kw in range(3):
                    for g in range(G):
                        wb[cg, kw, cc * HIN + r + kh, g * RT + r] = W9[kh, kw, ch, g]
    return xk, wb.astype(ml_dtypes.bfloat16)


def build_program(Tn, bvals):
    hs_bias = tuple(0.2 * float(v) + 0.5 for v in bvals)  # hard-sigmoid biases
    c_bias = float(bvals[2])
    nc = bacc.Bacc("TRN2", target_bir_lowering=False, debug=False)
    xk_d = nc.dram_tensor("xk", [Tn, KP, NCG, TAU, WP], BF16, kind="ExternalInput")
    wb_d = nc.dram_tensor("wb", [NCG, 3, KP, G * RT], BF16, kind="ExternalInput")
    out_d = nc.dram_tensor("out", [H, W], F32, kind="ExternalOutput")

    segs = h_window_segments()

    with tile.TileContext(nc) as tc:
        with tc.tile_pool(name="wpool", bufs=1) as wpool, \
             tc.tile_pool(name="xpool", bufs=3) as xpool, \
             tc.tile_pool(name="gpool", bufs=2) as gpool, \
             tc.tile_pool(name="tmpp", bufs=2) as tmpp, \
             tc.tile_pool(name="state", bufs=1) as state, \
             tc.tile_pool(name="zpsum", bufs=2, space="PSUM") as zpsum:

            # --- static weights / biases ---
            wt = [[wpool.tile([KP, G * RT], BF16, tag=f"w{cg}_{kw}",
                              name=f"w{cg}_{kw}")
                   for kw in range(3)] for cg in range(NCG)]
            for cg in range(NCG):
                for kw in range(3):
                    nc.sync.dma_start(out=wt[cg][kw], in_=wb_d[cg, kw])

            cbuf = state.tile([128, 2, W], F32, tag="cbuf", name="cbuf")
            nc.vector.memset(cbuf, 0.0)
            hbuf = state.tile([128, 2, WP], BF16, tag="hbuf", name="hbuf")
            nc.vector.memset(hbuf, 0.0)

            def load_x(t):
                xa = xpool.tile([KP, 2, TAU, WP], BF16, tag="xa", name="xa")
                xb = xpool.tile([KP, TAU, WP], BF16, tag="xb", name="xb")
                for cg in range(2):
                    nc.gpsimd.dma_start(out=xa[:, cg], in_=xk_d[t, :, cg])
                for half in range(2):
                    nc.gpsimd.dma_start(out=xb[:, 4 * half:4 * half + 4],
                                        in_=xk_d[t, :, 2, 4 * half:4 * half + 4])
                return xa, xb

            xt_cur = load_x(0)
            for t in range(Tn):
                xt_nxt = load_x(t + 1) if t + 1 < Tn else None

                # --- matmuls: 9 accumulating passes x 4 tau-pairs, N=512 ---
                xva = xt_cur[0].rearrange("p cg (b q) c -> p cg b q c", b=2)
                xvb = xt_cur[1].rearrange("p (b q) c -> p b q c", b=2)
                zt = [zpsum.tile([G * RT, 2, W], F32, tag=f"z{q}", name=f"z{q}")
                      for q in range(NPAIR)]
                gi = gpool.tile([128, 2, W], F32, tag="gi", name="gi")
                gf = gpool.tile([128, 2, W], F32, tag="gf", name="gf")
                go = gpool.tile([128, 2, W], F32, tag="go", name="go")
                sc = gpool.tile([128, 2, W], F32, tag="sc", name="sc")

                for cg in range(2):
                    for kw in range(3):
                        for q in range(NPAIR):
                            nc.tensor.matmul(
                                zt[q], wt[cg][kw],
                                xva[:, cg, :, q, kw:kw + W],
                                start=(cg == 0 and kw == 0), stop=False)
                for q in range(NPAIR):
                    for kw in range(3):
                        nc.tensor.matmul(
                            zt[q], wt[2][kw],
                            xvb[:, :, q, kw:kw + W],
                            start=False, stop=(kw == 2))
                    # fused deinterleave + gate activation for this pair:
                    # PSUM partitions g*32+r -> planar partitions q*32+r
                    sl = slice(q * RT, (q + 1) * RT)
                    for g_, dst in ((0, gi), (1, gf)):
                        nc.vector.tensor_scalar(
                            out=dst[sl], in0=zt[q][g_ * RT:(g_ + 1) * RT],
                            scalar1=0.2, scalar2=hs_bias[g_],
                            op0=OP.mult, op1=OP.add)
                    nc.scalar.activation(
                        out=go[sl], in_=zt[q][3 * RT:4 * RT], func=AF.Copy,
                        bias=hs_bias[3], scale=0.2)
                    nc.scalar.activation(
                        out=sc[sl], in_=zt[q][2 * RT:3 * RT], func=AF.Sigmoid,
                        bias=c_bias, scale=1.0)

                # --- clamp hard-sigmoid gates ---
                for dst in (gf, gi, go):
                    nc.vector.tensor_scalar(out=dst, in0=dst, scalar1=0.0,
                                            scalar2=1.0, op0=OP.max, op1=OP.min)

                # --- cell update ---
                t1 = tmpp.tile([128, 2, W], F32, tag="t1", name="t1")
                t2 = tmpp.tile([128, 2, W], F32, tag="t2", name="t2")
                nc.vector.tensor_tensor(out=t2, in0=gf, in1=cbuf, op=OP.mult)
                nc.vector.tensor_tensor(out=t1, in0=gi, in1=sc, op=OP.mult)
                nc.vector.tensor_tensor(out=cbuf, in0=t1, in1=t2, op=OP.add)
                s2 = tmpp.tile([128, 2, W], F32, tag="s2", name="s2")
                nc.scalar.activation(out=s2, in_=cbuf, func=AF.Sigmoid,
                                     bias=0.0, scale=1.0)

                # --- h = o * sigmoid(c), written straight into the next x
                # tile's h-channel halo windows (tau0/tau4 first: cg2 pair 0
                # of the next step depends only on those) ---
                if xt_nxt is not None:
                    nc.vector.tensor_tensor(out=hbuf[:, :, 1:W + 1], in0=go,
                                            in1=s2, op=OP.mult)
                    # scatter h into the next x tile's halo windows, pair-0
                    # taus first; spread issue over sync/scalar/gpsimd queues
                    order = sorted(range(len(segs)),
                                   key=lambda i: (segs[i][0] % 4, segs[i][0]))
                    engs = (nc.sync, nc.scalar)
                    for n, i in enumerate(order):
                        tau, s, e, p0, blk = segs[i]
                        engs[n % 2].dma_start(
                            out=xt_nxt[1][68 + s:68 + e, tau, :],
                            in_=hbuf[p0:p0 + (e - s), blk, :])
                else:
                    hf = tmpp.tile([128, 2, W], F32, tag="hf", name="hf")
                    nc.vector.tensor_tensor(out=hf, in0=go, in1=s2, op=OP.mult)
                    nc.sync.dma_start(
                        out=out_d.rearrange("(b p) w -> p b w", p=128),
                        in_=hf)
                xt_cur = xt_nxt
    nc.compile()
    return nc


_CACHE = {}


def _get_program(Tn, bvals):
    key = (Tn, bvals)
    if key not in _CACHE:
        _CACHE[key] = build_program(Tn, bvals)
    return _CACHE[key]


def kernel(x, Wx, Wh, b, _run_opts=None):
    x = np.asarray(x, dtype=np.float32)
    b = np.asarray(b, dtype=np.float32)
    Bn, Tn = x.shape[0], x.shape[1]
    xk, wb = pack_inputs(x, Wx, Wh)
    nc = _get_program(Tn, tuple(float(v) for v in b))
    in_maps = [{"xk": np.ascontiguousarray(xk[bi]), "wb": wb}
               for bi in range(Bn)]
    res = bass_utils.run_bass_kernel_spmd(
        nc, in_maps, core_ids=list(range(Bn)), **(_run_opts or {}))
    out = np.stack([res.results[bi]["out"] for bi in range(Bn)], axis=0)
    kernel.last_results = res
    return out[..., None].astype(np.float32)


# revision 9
# speedup vs baseline: 1.0137x; 1.0137x over previous
"""ConvLSTM2D forward on 8 Trainium2 NeuronCores (v3).

Problem: x [8,10,256,256,8], Wx [3,3,8,4], Wh [3,3,1,4], b [4]
         -> h_last [8,256,256,1]  (ConvLSTM, keras gate order i,f,c,o;
         i/f/o hard_sigmoid, candidate+output sigmoid)

Sharding: data-parallel over batch; core b computes batch element b fully
locally (recurrent scan stays on-core, no collectives in forward).

v3 structure (v2 was 374us, v1 420us):
 - bf16 matmul operands; x packed host-side as [T, 102, tau, cg, 258];
   4 load DMAs per step (3096B lines) on the sync queue.
 - 36 matmuls/step, N=512: pair (tau, tau+4) via 2-level free AP; 9
   accumulating passes into 4 PSUM banks (double buffered).
 - NO PSUM evacuation / deinterleave DMA: the PSUM->planar gate move is
   fused into the epilogue's first pass as partition-offset engine ops:
   per (pair q, gate): DVE/GpSimd tensor_scalar applies the hard-sigmoid
   affine (PSUM [32@g*32,2,256] -> planar bf16 [32@q*32,2,256]) and ACT
   applies Sigmoid for the candidate.  (v2 lost ~10us/step to a 256KB
   single-engine SBUF->SBUF deinterleave DMA on the critical path.)
 - NO h scatter DMA: h = o*sigmoid(c) is computed by DVE/GpSimd directly
   into the halo windows of the NEXT step's x tile (10 partition-offset
   tensor_tensor ops, windows tau0/tau4 first so cg2 matmuls restart
   early).
"""

import numpy as np
import ml_dtypes

import concourse.bacc as bacc
import concourse.bass as bass
import concourse.mybir as mybir
import concourse.tile as tile
from concourse import bass_utils

F32 = mybir.dt.float32
BF16 = mybir.dt.bfloat16
AF = mybir.ActivationFunctionType
OP = mybir.AluOpType

B, T, H, W, CIN = 8, 10, 256, 256, 8
G = 4            # gates i,f,c,o
RT = 32          # output rows per tile (M = G*RT = 128)
TAU = H // RT    # 8 row tiles
HIN = RT + 2     # input rows per tile (with halo)
NCH = 9          # 8 x-channels + h
CPG = 3          # channels per contraction group
NCG = NCH // CPG # 3 channel groups
KP = HIN * CPG   # 102 partitions per rhs tile
NPAIR = TAU // 2 # 4 tau-pairs (tau, tau+4) -> N=512 matmuls
WP = W + 2       # padded width


def h_window_segments():
    """(tau, seg_lo, seg_hi, planar_part0, planar_blk) for the h halo windows.

    Window rows for tau: 32*tau-1 .. 32*tau+32 (lr 0..33) at partition 68+lr;
    segments split where the window crosses the planar block boundary.
    """
    out = []
    for tau in range(TAU):
        r0 = tau * RT - 1
        lo = max(0, -r0)
        hi = min(HIN, H - r0)
        s = lo
        while s < hi:
            blk = (r0 + s) // 128
            e = min(hi, (blk + 1) * 128 - r0)
            out.append((tau, s, e, r0 + s - blk * 128, blk))
            s = e
    return out


def pack_inputs(x, Wx, Wh):
    """Host-side repack to bf16 device layouts.

    xk[b, t, cc*34+lr, cg, tau, 1+c] = x[b, t, 32*tau-1+lr, c, 3*cg+cc]
    wb[cg, kw, cc*34+r+kh, g*32+r]   = W9[kh, kw, 3*cg+cc, g]
    """
    x = np.asarray(x, dtype=np.float32)
    W9 = np.concatenate([np.asarray(Wx, np.float32),
                         np.asarray(Wh, np.float32)], axis=2)  # [3,3,9,4]

    xk = np.zeros((B, T, KP, NCG, TAU, WP), dtype=ml_dtypes.bfloat16)
    xb = x.astype(ml_dtypes.bfloat16)
    for tau in range(TAU):
        r0 = tau * RT - 1
        lo = max(0, -r0)
        hi = min(HIN, H - r0)
        for cg in range(NCG):
            for cc in range(CPG):
                ch = cg * CPG + cc
                if ch >= CIN:
                    continue  # h channel: written on device
                xk[:, :, cc * HIN + lo:cc * HIN + hi, cg, tau, 1:W + 1] = \
                    xb[:, :, r0 + lo:r0 + hi, :, ch]

    wb = np.zeros((NCG, 3, KP, G * RT), dtype=np.float32)
    r = np.arange(RT)
    for cg in range(NCG):
        for cc in range(CPG):
            ch = cg * CPG + cc
            for kh in range(3):
                for kw in range(3):
                    for g in range(G):
                        wb[cg, kw, cc * HIN + r + kh, g * RT + r] = W9[kh, kw, ch, g]
    return xk, wb.astype(ml_dtypes.bfloat16)


def build_program(Tn, bvals):
    hs_bias = tuple(0.2 * float(v) + 0.5 for v in bvals)  # hard-sigmoid biases
    c_bias = float(bvals[2])
    nc = bacc.Bacc("TRN2", target_bir_lowering=False, debug=False)
    xk_d = nc.dram_tensor("xk", [Tn, KP, NCG, TAU, WP], BF16, kind="ExternalInput")
    wb_d = nc.dram_tensor("wb", [NCG, 3, KP, G * RT], BF16, kind="ExternalInput")
    out_d = nc.dram_tensor("out", [H, W], F32, kind="ExternalOutput")

    segs = h_window_segments()

    with tile.TileContext(nc) as tc:
        with tc.tile_pool(name="wpool", bufs=1) as wpool, \
             tc.tile_pool(name="xpool", bufs=4) as xpool, \
             tc.tile_pool(name="gpool", bufs=2) as gpool, \
             tc.tile_pool(name="tmpp", bufs=2) as tmpp, \
             tc.tile_pool(name="state", bufs=1) as state, \
             tc.tile_pool(name="zpsum", bufs=2, space="PSUM") as zpsum:

            # --- static weights / biases ---
            wt = [[wpool.tile([KP, G * RT], BF16, tag=f"w{cg}_{kw}",
                              name=f"w{cg}_{kw}")
                   for kw in range(3)] for cg in range(NCG)]
            for cg in range(NCG):
                for kw in range(3):
                    nc.sync.dma_start(out=wt[cg][kw], in_=wb_d[cg, kw])

            cbuf = state.tile([128, 2, W], F32, tag="cbuf", name="cbuf")
            nc.vector.memset(cbuf, 0.0)
            hbuf = state.tile([128, 2, WP], BF16, tag="hbuf", name="hbuf")
            nc.vector.memset(hbuf, 0.0)

            def load_x(t):
                xa = xpool.tile([KP, 2, TAU, WP], BF16, tag="xa", name="xa")
                xb = xpool.tile([KP, TAU, WP], BF16, tag="xb", name="xb")
                for cg in range(2):
                    nc.sync.dma_start(out=xa[:, cg], in_=xk_d[t, :, cg])
                for half in range(2):
                    nc.sync.dma_start(out=xb[:, 4 * half:4 * half + 4],
                                      in_=xk_d[t, :, 2, 4 * half:4 * half + 4])
                return xa, xb

            xt_cur = load_x(0)
            for t in range(Tn):
                xt_nxt = load_x(t + 1) if t + 1 < Tn else None

                # --- matmuls: 9 accumulating passes x 4 tau-pairs, N=512 ---
                xva = xt_cur[0].rearrange("p cg (b q) c -> p cg b q c", b=2)
                xvb = xt_cur[1].rearrange("p (b q) c -> p b q c", b=2)
                zt = [zpsum.tile([G * RT, 2, W], F32, tag=f"z{q}", name=f"z{q}")
                      for q in range(NPAIR)]
                gi = gpool.tile([128, 2, W], F32, tag="gi", name="gi")
                gf = gpool.tile([128, 2, W], F32, tag="gf", name="gf")
                go = gpool.tile([128, 2, W], F32, tag="go", name="go")
                sc = gpool.tile([128, 2, W], F32, tag="sc", name="sc")

                for cg in range(2):
                    for kw in range(3):
                        for q in range(NPAIR):
                            nc.tensor.matmul(
                                zt[q], wt[cg][kw],
                                xva[:, cg, :, q, kw:kw + W],
                                start=(cg == 0 and kw == 0), stop=False)
                for q in range(NPAIR):
                    for kw in range(3):
                        nc.tensor.matmul(
                            zt[q], wt[2][kw],
                            xvb[:, :, q, kw:kw + W],
                            start=False, stop=(kw == 2))
                    # fused deinterleave + gate activation for this pair:
                    # PSUM partitions g*32+r -> planar partitions q*32+r
                    sl = slice(q * RT, (q + 1) * RT)
                    for g_, dst in ((0, gi), (1, gf)):
                        nc.vector.tensor_scalar(
                            out=dst[sl], in0=zt[q][g_ * RT:(g_ + 1) * RT],
                            scalar1=0.2, scalar2=hs_bias[g_],
                            op0=OP.mult, op1=OP.add)
                    nc.scalar.activation(
                        out=go[sl], in_=zt[q][3 * RT:4 * RT], func=AF.Copy,
                        bias=hs_bias[3], scale=0.2)
                    nc.scalar.activation(
                        out=sc[sl], in_=zt[q][2 * RT:3 * RT], func=AF.Sigmoid,
                        bias=c_bias, scale=1.0)

                # --- clamp hard-sigmoid gates ---
                for dst in (gf, gi, go):
                    nc.vector.tensor_scalar(out=dst, in0=dst, scalar1=0.0,
                                            scalar2=1.0, op0=OP.max, op1=OP.min)

                # --- cell update ---
                t1 = tmpp.tile([128, 2, W], F32, tag="t1", name="t1")
                t2 = tmpp.tile([128, 2, W], F32, tag="t2", name="t2")
                nc.vector.tensor_tensor(out=t2, in0=gf, in1=cbuf, op=OP.mult)
                nc.vector.tensor_tensor(out=t1, in0=gi, in1=sc, op=OP.mult)
                nc.vector.tensor_tensor(out=cbuf, in0=t1, in1=t2, op=OP.add)
                s2 = tmpp.tile([128, 2, W], F32, tag="s2", name="s2")
                nc.scalar.activation(out=s2, in_=cbuf, func=AF.Sigmoid,
                                     bias=0.0, scale=1.0)

                # --- h = o * sigmoid(c), written straight into the next x
                # tile's h-channel halo windows (tau0/tau4 first: cg2 pair 0
                # of the next step depends only on those) ---
                if xt_nxt is not None:
                    nc.vector.tensor_tensor(out=hbuf[:, :, 1:W + 1], in0=go,
                                            in1=s2, op=OP.mult)
                    # scatter h into the next x tile's halo windows, pair-0
                    # taus first; spread issue over sync/scalar/gpsimd queues
                    order = sorted(range(len(segs)),
                                   key=lambda i: (segs[i][0] % 4, segs[i][0]))
                    engs = (nc.sync, nc.scalar, nc.gpsimd)
                    for n, i in enumerate(order):
                        tau, s, e, p0, blk = segs[i]
                        engs[n % 3].dma_start(
                            out=xt_nxt[1][68 + s:68 + e, tau, :],
                            in_=hbuf[p0:p0 + (e - s), blk, :])
                else:
                    hf = tmpp.tile([128, 2, W], F32, tag="hf", name="hf")
                    nc.vector.tensor_tensor(out=hf, in0=go, in1=s2, op=OP.mult)
                    nc.sync.dma_start(
                        out=out_d.rearrange("(b p) w -> p b w", p=128),
                        in_=hf)
                xt_cur = xt_nxt
    nc.compile()
    return nc


_CACHE = {}


def _get_program(Tn, bvals):
    key = (Tn, bvals)
    if key not in _CACHE:
        _CACHE[key] = build_program(Tn, bvals)
    return _CACHE[key]


def kernel(x, Wx, Wh, b, _run_opts=None):
    x = np.asarray(x, dtype=np.float32)
    b = np.asarray(b, dtype=np.float32)
    Bn, Tn = x.shape[0], x.shape[1]
    xk, wb = pack_inputs(x, Wx, Wh)
    nc = _get_program(Tn, tuple(float(v) for v in b))
    in_maps = [{"xk": np.ascontiguousarray(xk[bi]), "wb": wb}
               for bi in range(Bn)]
    res = bass_utils.run_bass_kernel_spmd(
        nc, in_maps, core_ids=list(range(Bn)), **(_run_opts or {}))
    out = np.stack([res.results[bi]["out"] for bi in range(Bn)], axis=0)
    kernel.last_results = res
    return out[..., None].astype(np.float32)
